# revision 6
# baseline (speedup 1.0000x reference)
"""Trainium2 Bass kernel for segment min-max normalization.

Problem: x [1000000, 64] f32, seg [1000000] int32 (sorted, values in [0, 2048)).
  row_min/max over features, segment min/max over rows, then
  out = (x - seg_min[seg]) / (seg_max[seg] - seg_min[seg] + 1e-6).

Because seg is sorted, each segment is a contiguous row range, and the
per-segment scalar min/max equals the min/max over that contiguous block of x.

Sharding: segment-aligned. Core c owns segments [256c, 256c+256) == a
contiguous row range (~125k rows, padded to 131072). Fully local per core.

Per-core layout: local row = p*1024 + u*64 + r  (partition p<128, sub-tile
u<16, row-in-block r<64). Each (p,u) 64-row block intersects at most 2
segments (min segment length ~416 >> 64); each partition's 1024 rows span at
most ~4 segments, so a 16-wide window of per-segment values per partition
covers every block's candidates.

Pipeline (single NEFF, SPMD across 8 cores):
  1. Stream x in 2MB tiles ([128 x 64 x 64], per-partition contiguous 16KB),
     reduce innermost -> per-row min/max; one contiguous DMA writes the
     row-ordered stats to DRAM scratch.
  2. dma_gather one 704-row stat window per segment (window start = 64-aligned
     segment start, host-built int16 indices), +-1e30 host mask, reduce ->
     per-segment min/max [256]. scale = 1/(max-min+eps) (Newton-refined
     reciprocal), bias = -min*scale; (bias,scale) pairs -> DRAM.
  3. One [128,1]-indexed indirect DMA pulls each partition's 16-pair window
     pairs[wbase[p]:wbase[p]+16]. Host-built one-hots select each block's A/B
     candidate pairs; host-built per-block cut + rowpos select per-row
     (bias,scale) = mask*(A-B) + B.
  4. Stream x again; out = x*scale + bias (broadcast over features); store.

All seg-derived index tensors are built on the host (seg is a kernel input),
so the compiled program is input-independent. Scratch lives in ExternalOutput
tensors: Internal DRAM scratch is NOT private per NeuronCore here (cores
sharing an HBM stack clobber each other's static scratch allocations).
"""

from contextlib import ExitStack

import numpy as np

N_CORES = 8
NUM_SEGMENTS = 2048
SEGS_PER_CORE = NUM_SEGMENTS // N_CORES  # 256
D = 64
R = 131072  # padded rows per core
RPP = 64  # rows per block
NSUB = 16  # blocks (sub-tiles) per partition
RPP_PART = NSUB * RPP  # rows per partition (1024)
NTILES = NSUB  # one mega-tile per sub-tile index
WIN = 704  # gather window (rows) per segment; needs >= 64 + max seg len
NWIN = (R - WIN) // RPP + 1
JWIN = 16  # per-partition pair-window width (segments)
EPS = 1e-6
BIG = 1e30

_CACHE = {}


def _build_program():
    import concourse.bass as bass
    import concourse.tile as tile
    from concourse import bacc, mybir

    f32 = mybir.dt.float32
    AX = mybir.AxisListType
    OP = mybir.AluOpType

    nc = bacc.Bacc("TRN2", target_bir_lowering=False, debug=False)

    x_d = nc.dram_tensor("x", [R, D], f32, kind="ExternalInput")
    gidx_d = nc.dram_tensor("gidx", [128, 16], mybir.dt.int16, kind="ExternalInput")
    maskbig_d = nc.dram_tensor("maskbig", [128, 2, WIN], f32, kind="ExternalInput")
    wbase_d = nc.dram_tensor("wbase", [128, 1], mybir.dt.int32, kind="ExternalInput")
    onehota_d = nc.dram_tensor(
        "onehota", [128, NSUB, JWIN], f32, kind="ExternalInput"
    )
    onehotb_d = nc.dram_tensor(
        "onehotb", [128, NSUB, JWIN], f32, kind="ExternalInput"
    )
    cut_d = nc.dram_tensor("cut", [128, NSUB], f32, kind="ExternalInput")
    rowpos_d = nc.dram_tensor("rowpos", [128, RPP], f32, kind="ExternalInput")
    y_d = nc.dram_tensor("y", [R, D], f32, kind="ExternalOutput")

    # Scratch as ExternalOutput: per-core-private buffers (Internal DRAM
    # scratch is shared/racy between cores on the same HBM stack).
    smin_d = nc.dram_tensor("smin", [R], f32, kind="ExternalOutput")
    smax_d = nc.dram_tensor("smax", [R], f32, kind="ExternalOutput")
    pairs_d = nc.dram_tensor("pairs", [SEGS_PER_CORE, 2], f32, kind="ExternalOutput")

    x_view = x_d.ap().rearrange("(p u r) d -> p u r d", p=128, u=NSUB)
    y_view = y_d.ap().rearrange("(p u r) d -> p u r d", p=128, u=NSUB)

    with tile.TileContext(nc) as tc, ExitStack() as ctx:
        xpool = ctx.enter_context(tc.tile_pool(name="x", bufs=4))
        meta = ctx.enter_context(tc.tile_pool(name="meta", bufs=1))

        stats_min = meta.tile([128, NSUB, RPP], f32)
        stats_max = meta.tile([128, NSUB, RPP], f32)

        # ---- pass A: per-row min/max ----
        for u in range(NSUB):
            xt = xpool.tile([128, RPP, D], f32, tag="xt")
            nc.sync.dma_start(out=xt[:], in_=x_view[:, u])
            nc.vector.tensor_reduce(
                out=stats_min[:, u, :], in_=xt[:], axis=AX.X, op=OP.min
            )
            nc.vector.tensor_reduce(
                out=stats_max[:, u, :], in_=xt[:], axis=AX.X, op=OP.max
            )

        sview_min = smin_d.ap().rearrange("(p q) -> p q", p=128)
        sview_max = smax_d.ap().rearrange("(p q) -> p q", p=128)
        nc.sync.dma_start(out=sview_min, in_=stats_min[:])
        nc.sync.dma_start(out=sview_max, in_=stats_max[:])

        # ---- segment reduce via windowed gather ----
        gidx_sb = meta.tile([128, 16], mybir.dt.int16)
        nc.sync.dma_start(out=gidx_sb[:], in_=gidx_d.ap())
        mb_sb = meta.tile([128, 2, WIN], f32)
        nc.sync.dma_start(out=mb_sb[:], in_=maskbig_d.ap())

        win_min = meta.tile([128, 2, WIN], f32)
        win_max = meta.tile([128, 2, WIN], f32)
        wmin_ap = bass.AP(tensor=smin_d, offset=0, ap=[[RPP, NWIN], [1, WIN]])
        wmax_ap = bass.AP(tensor=smax_d, offset=0, ap=[[RPP, NWIN], [1, WIN]])
        nc.gpsimd.dma_gather(
            out_ap=win_min[:],
            in_ap=wmin_ap,
            idxs_ap=gidx_sb[:],
            num_idxs=SEGS_PER_CORE,
            num_idxs_reg=SEGS_PER_CORE,
            elem_size=WIN,
            elem_step=RPP,
        )
        nc.gpsimd.dma_gather(
            out_ap=win_max[:],
            in_ap=wmax_ap,
            idxs_ap=gidx_sb[:],
            num_idxs=SEGS_PER_CORE,
            num_idxs_reg=SEGS_PER_CORE,
            elem_size=WIN,
            elem_step=RPP,
        )

        nc.vector.tensor_tensor(out=win_min[:], in0=win_min[:], in1=mb_sb[:], op=OP.add)
        nc.vector.tensor_tensor(
            out=win_max[:], in0=win_max[:], in1=mb_sb[:], op=OP.subtract
        )
        g_min = meta.tile([128, 2], f32)
        g_max = meta.tile([128, 2], f32)
        nc.vector.tensor_reduce(out=g_min[:], in_=win_min[:], axis=AX.X, op=OP.min)
        nc.vector.tensor_reduce(out=g_max[:], in_=win_max[:], axis=AX.X, op=OP.max)

        denom = meta.tile([128, 2], f32)
        nc.vector.tensor_tensor(
            out=denom[:], in0=g_max[:], in1=g_min[:], op=OP.subtract
        )
        nc.vector.tensor_scalar_add(out=denom[:], in0=denom[:], scalar1=EPS)

        # scale = 1/denom, Newton-refined: s1 = s0*(2 - d*s0), twice.
        scale = meta.tile([128, 2], f32)
        tmp = meta.tile([128, 2], f32)
        nc.vector.reciprocal(out=scale[:], in_=denom[:])
        for _ in range(2):
            nc.vector.tensor_tensor(out=tmp[:], in0=denom[:], in1=scale[:], op=OP.mult)
            nc.vector.tensor_scalar(
                out=tmp[:],
                in0=tmp[:],
                scalar1=-1.0,
                scalar2=2.0,
                op0=OP.mult,
                op1=OP.add,
            )
            nc.vector.tensor_tensor(out=scale[:], in0=scale[:], in1=tmp[:], op=OP.mult)

        bias = meta.tile([128, 2], f32)
        nc.vector.tensor_scalar_mul(out=bias[:], in0=g_min[:], scalar1=-1.0)
        nc.vector.tensor_tensor(out=bias[:], in0=bias[:], in1=scale[:], op=OP.mult)

        pairs = meta.tile([128, 2, 2], f32)
        nc.vector.tensor_copy(out=pairs[:, :, 0:1], in_=bias[:].unsqueeze(2))
        nc.vector.tensor_copy(out=pairs[:, :, 1:2], in_=scale[:].unsqueeze(2))
        pview = pairs_d.ap().rearrange("(c p) k -> p c k", p=128)
        nc.sync.dma_start(out=pview, in_=pairs[:])

        # ---- per-partition pair window + per-block one-hot select ----
        wbase_sb = meta.tile([128, 1], mybir.dt.int32)
        nc.sync.dma_start(out=wbase_sb[:], in_=wbase_d.ap())
        oha_sb = meta.tile([128, NSUB, JWIN], f32)
        ohb_sb = meta.tile([128, NSUB, JWIN], f32)
        nc.sync.dma_start(out=oha_sb[:], in_=onehota_d.ap())
        nc.sync.dma_start(out=ohb_sb[:], in_=onehotb_d.ap())
        cut_sb = meta.tile([128, NSUB], f32)
        rowpos_sb = meta.tile([128, RPP], f32)
        nc.sync.dma_start(out=cut_sb[:], in_=cut_d.ap())
        nc.sync.dma_start(out=rowpos_sb[:], in_=rowpos_d.ap())

        # NOTE: the indirect-DMA walrus lowering only supports a 2D SBUF out
        # (K contiguous elements per partition); a 3D out silently fills only
        # partition 0 on hardware.
        awin = meta.tile([128, JWIN * 2], f32)
        nc.gpsimd.indirect_dma_start(
            out=awin[:],
            out_offset=None,
            in_=pairs_d.ap(),
            in_offset=bass.IndirectOffsetOnAxis(ap=wbase_sb[:], axis=0),
        )

        # A/B candidate pair per block: contract window along j with one-hots.
        awin_t = awin[:].rearrange("p (j k) -> p k j", k=2)  # [128, 2, JWIN]
        tmp4 = meta.tile([128, NSUB, 2, JWIN], f32)
        a_sb = meta.tile([128, NSUB, 2], f32)
        b_sb = meta.tile([128, NSUB, 2], f32)
        nc.vector.tensor_tensor(
            out=tmp4[:],
            in0=oha_sb[:].unsqueeze(2).to_broadcast([128, NSUB, 2, JWIN]),
            in1=awin_t.unsqueeze(1).to_broadcast([128, NSUB, 2, JWIN]),
            op=OP.mult,
        )
        nc.vector.tensor_reduce(out=a_sb[:], in_=tmp4[:], axis=AX.X, op=OP.add)
        nc.vector.tensor_tensor(
            out=tmp4[:],
            in0=ohb_sb[:].unsqueeze(2).to_broadcast([128, NSUB, 2, JWIN]),
            in1=awin_t.unsqueeze(1).to_broadcast([128, NSUB, 2, JWIN]),
            op=OP.mult,
        )
        nc.vector.tensor_reduce(out=b_sb[:], in_=tmp4[:], axis=AX.X, op=OP.add)

        mask = meta.tile([128, NSUB, RPP], f32)
        nc.vector.tensor_tensor(
            out=mask[:],
            in0=rowpos_sb[:].unsqueeze(1).to_broadcast([128, NSUB, RPP]),
            in1=cut_sb[:].unsqueeze(2).to_broadcast([128, NSUB, RPP]),
            op=OP.is_lt,
        )
        amb = meta.tile([128, NSUB, 2], f32)
        nc.vector.tensor_tensor(out=amb[:], in0=a_sb[:], in1=b_sb[:], op=OP.subtract)
        mn_inv = meta.tile([128, NSUB, RPP, 2], f32)
        nc.vector.tensor_tensor(
            out=mn_inv[:],
            in0=mask[:].unsqueeze(3).to_broadcast([128, NSUB, RPP, 2]),
            in1=amb[:].unsqueeze(2).to_broadcast([128, NSUB, RPP, 2]),
            op=OP.mult,
        )
        nc.vector.tensor_tensor(
            out=mn_inv[:],
            in0=mn_inv[:],
            in1=b_sb[:].unsqueeze(2).to_broadcast([128, NSUB, RPP, 2]),
            op=OP.add,
        )

        # ---- pass B: normalize ----
        for u in range(NSUB):
            xt = xpool.tile([128, RPP, D], f32, tag="xt")
            nc.sync.dma_start(out=xt[:], in_=x_view[:, u])
            scale_ap = mn_inv[:, u, :, 1:2].to_broadcast([128, RPP, D])
            bias_ap = mn_inv[:, u, :, 0:1].to_broadcast([128, RPP, D])
            nc.vector.tensor_tensor(out=xt[:], in0=xt[:], in1=scale_ap, op=OP.mult)
            nc.vector.tensor_tensor(out=xt[:], in0=xt[:], in1=bias_ap, op=OP.add)
            nc.sync.dma_start(out=y_view[:, u], in_=xt[:])

    nc.compile()
    return nc


def get_program():
    if "nc" not in _CACHE:
        _CACHE["nc"] = _build_program()
    return _CACHE["nc"]


def build_core_inputs(x, seg):
    """Host-side prep: shard rows segment-aligned, build all index tensors.

    Returns (in_maps, bounds) where bounds[c] = (row_start, row_end)."""
    N = x.shape[0]
    B = np.searchsorted(seg, np.arange(NUM_SEGMENTS + 1)).astype(np.int64)
    assert B[-1] == N

    rowpos = np.broadcast_to(
        np.arange(RPP, dtype=np.float32)[None, :], (128, RPP)
    ).copy()

    in_maps = []
    bounds = []
    for c in range(N_CORES):
        s0 = c * SEGS_PER_CORE
        b0, b1 = int(B[s0]), int(B[s0 + SEGS_PER_CORE])
        rc = b1 - b0
        assert rc <= R, f"core {c} shard {rc} rows exceeds padded size {R}"
        bounds.append((b0, b1))

        # row-order shard, then permute rows into the device layout:
        # device row (p, u, r) = local row p*1024 + u*64 + r  (identity here;
        # the layout IS row-order, just tiled).
        xs = np.zeros((R, D), dtype=np.float32)
        xs[:rc] = x[b0:b1]

        a = B[s0 : s0 + SEGS_PER_CORE] - b0  # local segment starts [256]
        ln = np.diff(B[s0 : s0 + SEGS_PER_CORE + 1])  # lengths [256]
        wstart = (a // RPP).astype(np.int16)
        off = (a - wstart.astype(np.int64) * RPP).astype(np.int64)
        assert int((off + ln).max() if len(ln) else 0) <= WIN

        # gidx: gather index i lives at [i % 16, i // 16], replicated to 128
        gidx = np.zeros((16, 16), dtype=np.int16)
        i = np.arange(SEGS_PER_CORE)
        gidx[i % 16, i // 16] = wstart
        gidx = np.tile(gidx, (8, 1))

        # maskbig[i%128, i//128, j] = 0 where valid (off<=j<off+len) else BIG
        j = np.arange(WIN)[None, :]
        invalid = (j < off[:, None]) | (j >= (off + ln)[:, None])
        maskbig = np.zeros((128, 2, WIN), dtype=np.float32)
        maskbig[i % 128, i // 128, :] = invalid.astype(np.float32) * BIG

        # per-block candidate segments; block k = p*NSUB + u, rows k*64..+64
        nblk = R // RPP
        bstart = np.arange(nblk, dtype=np.int64) * RPP
        bend = bstart + RPP - 1
        Bl = B[s0 : s0 + SEGS_PER_CORE + 1] - b0  # local boundaries [257]
        clip_s = np.clip(bstart, 0, max(rc - 1, 0))
        clip_e = np.clip(bend, 0, max(rc - 1, 0))
        ia = np.searchsorted(Bl, clip_s, side="right") - 1
        ib = np.searchsorted(Bl, clip_e, side="right") - 1
        ia = np.clip(ia, 0, SEGS_PER_CORE - 1)
        ib = np.clip(ib, 0, SEGS_PER_CORE - 1)
        cut = np.where(ia == ib, RPP, Bl[ia + 1] - bstart).astype(np.float64)
        cut = np.clip(cut, 0, RPP)

        iab = ia.reshape(128, NSUB)
        ibb = ib.reshape(128, NSUB)
        wbase = np.minimum(iab[:, 0], SEGS_PER_CORE - JWIN).astype(np.int32)
        ja = iab - wbase[:, None]
        jb = ibb - wbase[:, None]
        assert ja.min() >= 0 and ja.max() < JWIN, (ja.min(), ja.max())
        assert jb.min() >= 0 and jb.max() < JWIN, (jb.min(), jb.max())
        jj = np.arange(JWIN)
        onehota = (ja[:, :, None] == jj).astype(np.float32)
        onehotb = (jb[:, :, None] == jj).astype(np.float32)

        in_maps.append(
            {
                "x": xs,
                "gidx": gidx,
                "maskbig": maskbig,
                "wbase": wbase[:, None],
                "onehota": onehota,
                "onehotb": onehotb,
                "cut": cut.reshape(128, NSUB).astype(np.float32),
                "rowpos": rowpos,
            }
        )
    return in_maps, bounds


def _get_runner():
    """Build (once) a cached jitted SPMD runner mirroring
    bass2jax.run_bass_via_pjrt's multi-core path, so repeat kernel() calls
    skip retracing/recompiling."""
    if "runner" in _CACHE:
        return _CACHE["runner"]

    import jax
    import jax.numpy as jnp
    from jax.sharding import Mesh, PartitionSpec
    from jax.experimental.shard_map import shard_map
    from concourse import bass2jax, mybir

    nc = get_program()
    bass2jax.install_neuronx_cc_hook()

    partition_name = (
        nc.partition_id_tensor.name if nc.partition_id_tensor is not None else None
    )
    in_names, out_names, out_avals, zero_shapes = [], [], [], []
    for alloc in nc.m.functions[0].allocations:
        if not isinstance(alloc, mybir.MemoryLocationSet):
            continue
        name = alloc.memorylocations[0].name
        if alloc.kind == "ExternalInput":
            if name != partition_name:
                in_names.append(name)
        elif alloc.kind == "ExternalOutput":
            out_names.append(name)
            shape = tuple(alloc.tensor_shape)
            dtype = mybir.dt.np(alloc.dtype)
            out_avals.append(jax.core.ShapedArray(shape, dtype))
            zero_shapes.append((shape, dtype))
    n_params = len(in_names)
    all_names = list(in_names) + list(out_names)
    if partition_name is not None:
        all_names.append(partition_name)
    donate = tuple(range(n_params, n_params + len(out_names)))

    def _body(*args):
        operands = list(args)
        if partition_name is not None:
            operands.append(bass2jax.partition_id_tensor())
        outs = bass2jax._bass_exec_p.bind(
            *operands,
            out_avals=tuple(out_avals),
            in_names=tuple(all_names),
            out_names=tuple(out_names),
            lowering_input_output_aliases=(),
            sim_require_finite=True,
            sim_require_nnan=True,
            nc=nc,
        )
        return tuple(outs)

    devices = jax.devices()[:N_CORES]
    mesh = Mesh(np.asarray(devices), ("core",))
    nin = n_params + len(out_names)
    sharded = jax.jit(
        shard_map(
            _body,
            mesh=mesh,
            in_specs=(PartitionSpec("core"),) * nin,
            out_specs=(PartitionSpec("core"),) * len(out_names),
            check_rep=False,
        ),
        donate_argnums=donate,
        keep_unused=True,
    )

    def run(in_maps):
        concat_in = [
            np.concatenate([np.asarray(m[name]) for m in in_maps], axis=0)
            for name in in_names
        ]
        concat_zeros = [
            np.zeros((N_CORES * s[0], *s[1:]), d) for s, d in zero_shapes
        ]
        out_arrs = sharded(*concat_in, *concat_zeros)
        yi = out_names.index("y")
        return np.asarray(out_arrs[yi]).reshape(N_CORES, R, D)

    _CACHE["runner"] = run
    return run


def kernel(**inputs):
    x = np.ascontiguousarray(np.asarray(inputs["x"], dtype=np.float32))
    seg = np.asarray(inputs["seg"], dtype=np.int64)
    in_maps, bounds = build_core_inputs(x, seg)
    try:
        y = _get_runner()(in_maps)
        out = np.empty_like(x)
        for c, (b0, b1) in enumerate(bounds):
            out[b0:b1] = y[c][: b1 - b0]
        return out
    except Exception:
        from concourse.bass_utils import run_bass_kernel_spmd

        res = run_bass_kernel_spmd(
            get_program(), in_maps, core_ids=list(range(N_CORES))
        )
        out = np.empty_like(x)
        for c, (b0, b1) in enumerate(bounds):
            out[b0:b1] = res.results[c]["y"][: b1 - b0]
        return out
